# revision 122
# baseline (speedup 1.0000x reference)
"""DeBERTa-style disentangled attention head for Trainium2 (Bass/Tile).

Problem: B=8, S=2048, D_MODEL=1024, D_HEAD=64, K2=2048.
Strategy: data-parallel over batch across 8 NeuronCores; per core a
transposed-attention formulation:
  scoresT[j, i] = c2c + c2p + p2c gathered via skew (diagonal-AP) DMAs,
  unsafe softmax (no max subtraction; scores are O(1)), denominator via a
  ones-column folded into the AV matmul, final small transpose.

Phase A: x/pos_x loaded as bf16 via casting SWDGE mega-loads (8 blocks per
DMA; per-DMA launch/sem overhead dominates the Pool pipe), bf16 PE
transposes into a 2-window ring, bf16 projections (weight stacks pack two
64-wide outputs per 128); pos_x first — x slices 0-1 transpose during
pos_x's DMA-starved tail — then x interleaved with the full-width c2p
strips (q . kr_rev, computed once per i-tile, bf16, 4x-penalty-free
448-wide chunks).
Phase B: per j-block pair a single skew gather pulls c2p diagonals for 8
i-tiles with 512B lines; p2c strips gathered per j-block (2KB lines, SWDGE);
score PSUM accumulates c2c (fp32r) + transpose-injects (bf16 stationary x
identity) + p2c injects; exp (with mask bias) lands in a 2x4-block bf16
ring; AV chains 4 j-blocks per PSUM accumulation group. Everything is
software-pipelined ~2 iterations ahead on separate ring tiles (precise
per-tile dependency tracking; a single shared scratch degrades the tile
dep tracker and serializes).
"""
import numpy as np
import bass_rust
import concourse.bass as bass
import concourse.mybir as mybir
import concourse.tile as tile
from concourse.tile import ScopedClock

B, S, D, DH, K2 = 8, 2048, 1024, 64, 2048
NB = S // 128            # 16 blocks of 128
KC = D // 128            # 8 contraction chunks
WSTR = 2176              # full strip width per i-tile (bf16 elems)
WSROW = NB * WSTR        # strips tile row width (34816 bf16)

CH5 = ((0, 448), (448, 448), (896, 448), (1344, 448), (1792, 384))
SCALE = float(1.0 / np.sqrt(3 * DH))
NEG = -1e30

F32 = mybir.dt.float32
F32R = mybir.dt.float32r
BF16 = mybir.dt.bfloat16
I32 = mybir.dt.int32

AFT = mybir.ActivationFunctionType
ALU = mybir.AluOpType

# ---------------------------------------------------------------- patches ---

_nop_counter = [0]


def _drain_and_barrier_split(self, tick_clock, wait_clock):
    nc = self.nc
    drain_inst = nc.sync.drain()
    wait_clock.add_sem_waits(
        drain_inst.ins, ScopedClock({None: tick_clock.global_clock})
    )
    si = drain_inst.ins.sync_info
    waits = list(si.on_wait) if si is not None and si.on_wait else []
    if len(waits) > 1:
        drain_inst.ins.sync_info = bass_rust.SyncInfo(
            on_wait=[waits[0]], on_update=list(si.on_update or [])
        )
        for w in waits[1:]:
            d2 = nc.sync.drain()
            d2.ins.sync_info = bass_rust.SyncInfo(on_wait=[w], on_update=[])
    nc.all_engine_barrier()
    assert self.sems is not None
    popped = nc._tile_sem_poison_stack.pop()
    assert popped is self._sem_poison
    nc.clear_and_free_semaphores(list(self.sems.allocated().values()))
    nc.all_engine_barrier()


def _split_excess_waits(nc):
    MAXW = {"EventSemaphore": 2}
    for f in nc.m.functions:
        new_blocks = []
        changed = False
        for bb in f.blocks:
            insts = list(bb.instructions)
            new_insts = []
            bb_changed = False
            for inst in insts:
                si = inst.sync_info
                waits = list(si.on_wait) if si is not None and si.on_wait else []
                cap = MAXW.get(str(inst.opcode), 1)
                if len(waits) > cap:
                    for w in waits[cap:]:
                        _nop_counter[0] += 1
                        nop = bass_rust.InstNoOp(
                            name=f"I-waitsplit-{_nop_counter[0]}", ins=[], outs=[]
                        )
                        nop.engine = inst.engine
                        nop.sync_info = bass_rust.SyncInfo(on_wait=[w], on_update=[])
                        new_insts.append(nop)
                    inst.sync_info = bass_rust.SyncInfo(
                        on_wait=waits[:cap], on_update=list(si.on_update or [])
                    )
                    bb_changed = True
                new_insts.append(inst)
            if bb_changed:
                nb = bass_rust.BasicBlock(name=bb.name, instructions=new_insts)
                nb.IsExit = bb.IsExit
                nb.IsLoopEntry = bb.IsLoopEntry
                nb.IsPredicated = bb.IsPredicated
                new_blocks.append(nb)
                changed = True
            else:
                new_blocks.append(bb)
        if changed:
            f.blocks = new_blocks


tile.TileContext._drain_and_barrier = _drain_and_barrier_split

# ------------------------------------------------------------- AP helpers ---


def _set_ap(ap, pairs, offset=None):
    v = ap.ap
    v.clear()
    for p in pairs:
        v.append(tuple(int(z) for z in p))
    ap.ap = v
    if offset is not None:
        ap.offset = int(offset)
    return ap


def rev_free(ap, n):
    """Reverse the (single) free dim of a 2D AP of width n."""
    out = ap.copy()
    pairs = list(out.ap)
    assert len(pairs) == 2
    (pstep, pcount), (fstep, fcount) = pairs
    assert fstep == 1 and fcount == n
    return _set_ap(out, [(pstep, pcount), (-1, n)], out.offset + n - 1)


def _make_identity(nc, ident):
    nc.gpsimd.memset(ident, 0.0)
    nc.gpsimd.affine_select(
        out=ident,
        in_=ident,
        compare_op=ALU.not_equal,
        fill=1.0,
        base=0,
        pattern=[[-1, ident.shape[0]]],
        channel_multiplier=1,
    )


# ----------------------------------------------------------------- build ----


def build_nc(split_waits=True, variant="full", reps=1, debug=False):
    nc = bass.Bass()
    x_d = nc.dram_tensor("x", [S, D], F32, kind="ExternalInput")
    px_d = nc.dram_tensor("pos_x", [K2, D], F32, kind="ExternalInput")
    mask_d = nc.dram_tensor("padding_mask", [S], I32, kind="ExternalInput")
    w_d = {}
    for w in ("Wq", "Wk", "Wv", "Wqr", "Wkr"):
        w_d[w] = nc.dram_tensor(w, [D, DH], F32, kind="ExternalInput")
    b_d = {}
    for b in ("bq", "bk", "bv", "bqr", "bkr"):
        b_d[b] = nc.dram_tensor(b, [DH], F32, kind="ExternalInput")
    out_d = nc.dram_tensor("out", [S, DH], F32, kind="ExternalOutput")

    with tile.TileContext(nc) as tc:
        with (
            tc.tile_pool(name="consts", bufs=1) as consts,
            tc.tile_pool(name="big", bufs=1) as big,
        ):
            ident32 = consts.tile([128, 128], F32, name="ident32")
            identb = consts.tile([128, 128], BF16, name="identb")
            _make_identity(nc, ident32[:])
            nc.vector.tensor_copy(identb[:], ident32[:])

            # persistent operand buffers
            buf1 = big.tile([128, S], F32R, name="buf1")      # [KT; KT]
            buf2 = big.tile([128, 4096], F32R, name="buf2")   # [QT pad; QRT pad]
            buf4 = big.tile([64, 4096], F32R, name="buf4")    # KRT' rev pad
            vtil = big.tile([128, NB * (DH + 1)], BF16, name="vtil")
            outacc = big.tile([65, S], F32, name="outacc")
            strips = big.tile([128, WSROW], BF16, name="strips")
            outsb = big.tile([128, NB * DH], F32, name="outsb")
            cpv = [
                big.tile([128, 2048], BF16, name=f"cpv{i}") for i in range(3)
            ]

            maskb = consts.tile([128, NB], F32, name="maskb")
            ones_col = consts.tile([128, 1], BF16, name="ones_col")
            nc.vector.memset(ones_col[:], 1.0)

            for _rep in range(reps):
                # ======================= phase A ==========================
                with (
                    tc.tile_pool(name="apool", bufs=1) as apool,
                    tc.tile_pool(name="ldpool", bufs=2) as ldpool,
                    tc.tile_pool(name="wldpool", bufs=3) as wldpool,
                    tc.tile_pool(name="tpsum", bufs=3, space="PSUM") as tpsum,
                    tc.tile_pool(name="ppsum", bufs=2, space="PSUM") as ppsum,
                    tc.tile_pool(name="stpsum", bufs=3, space="PSUM") as stpsum,
                ):
                    xtb = apool.tile([128, 8192], BF16, name="xtb")
                    vtsb = apool.tile([64, S], BF16, name="vtsb")
                    wsv = apool.tile([128, 2560], BF16, name="wsv")

                    # ---- x: one casting load (f32 -> bf16, SWDGE) per
                    # 512-row slice, then bf16 PE transposes into the ring.
                    # pos_x: f32 half-quad loads on the two HWDGE queues with
                    # f32 transposes (evictions round to bf16 either way).
                    def load_octo(src_dram, half):
                        xt8 = ldpool.tile([128, 8192], BF16, name="xo", tag="xld")
                        dst = _set_ap(
                            xt8[:].copy(), [(8192, 128), (1024, 8), (1, 1024)], 0
                        )
                        srcv = src_dram[:].copy()
                        _set_ap(
                            srcv, [(D, 128), (128 * D, 8), (1, D)], half * 1024 * D
                        )
                        nc.gpsimd.dma_start(dst, srcv)
                        return xt8

                    def _tp_evict(o_ap, ps, par):
                        if par == 0:
                            nc.vector.tensor_copy(o_ap, ps[:])
                        else:
                            nc.scalar.activation(o_ap, ps[:], AFT.Copy)

                    def _xtb_ap(ib, g):
                        wb = ((ib // 4) % 2) * 4096
                        o = xtb[:].copy()
                        return _set_ap(
                            o,
                            [(8192, 128), (512, 4), (1, 128)],
                            wb + (g * 4) * 512 + (ib % 4) * 128,
                        )

                    def transpose_quad(xt4, ib, ibq):
                        for g in range(2):
                            ps = tpsum.tile([128, 512], BF16, name="tps", tag="tps")
                            for q in range(4):
                                kc = g * 4 + q
                                nc.tensor.matmul(
                                    ps[:, q * 128 : (q + 1) * 128],
                                    xt4[:, ibq * 1024 + kc * 128 : ibq * 1024 + (kc + 1) * 128],
                                    identb[:],
                                    is_transpose=True,
                                    skip_group_check=True,
                                )
                            _tp_evict(_xtb_ap(ib, g), ps, (ib + g) % 2)

                    # ---- weights: strided DMA into a small f32 ring, then
                    # converting copies (f32 -> bf16) into the weight stacks
                    wst = {"P1": 0, "P3": 1024, "P4": 2048}

                    def load_w(w):
                        wt = wldpool.tile([128, KC * DH], F32, name=f"wl_{w}", tag="wld")
                        dst = _set_ap(
                            wt[:].copy(), [(KC * DH, 128), (DH, KC), (1, DH)], 0
                        )
                        srcv = w_d[w][:].copy()
                        _set_ap(srcv, [(DH, 128), (128 * DH, KC), (1, DH)], 0)
                        nc.scalar.dma_start(dst, srcv)
                        return wt

                    def stack_half(wt, sbase, half):
                        o = wsv[:].copy()
                        step = 128 if half is not None else DH
                        _set_ap(
                            o, [(2560, 128), (step, KC), (1, DH)],
                            sbase + (half or 0) * DH,
                        )
                        i = _set_ap(
                            wt[:].copy(), [(KC * DH, 128), (DH, KC), (1, DH)], 0
                        )
                        nc.vector.tensor_copy(o, i)

                    # first pos_x mega-load, then only the weights A1 needs
                    # (P3); the rest load mid-A1 to keep the DMA pipe clear
                    M = [None] * 4
                    M[0] = load_octo(px_d, 0)
                    M[1] = load_octo(px_d, 1)
                    for half, w in enumerate(("Wkr", "Wqr")):
                        stack_half(load_w(w), wst["P3"], half)

                    # biases (tiny; needed by the first projection evictions)
                    bstack = {}
                    for sname, (ba, bb) in {
                        "P1": ("bq", "bk"),
                        "P3": ("bkr", "bqr"),
                    }.items():
                        bt = consts.tile([128, 1], F32, name=f"bs_{sname}")
                        nc.scalar.dma_start(bt[0:64, :], b_d[ba][:])
                        nc.scalar.dma_start(bt[64:128, :], b_d[bb][:])
                        bstack[sname] = bt
                    bt4 = consts.tile([64, 1], F32, name="bs_P4")
                    nc.scalar.dma_start(bt4[:], b_d["bv"][:])
                    bstack["P4"] = bt4

                    def project_slice(sname, nsl, evict_fn, mwid):
                        wb = (nsl % 2) * 4096
                        sb = wst[sname]
                        ps = ppsum.tile([128, 512], F32, name="pps", tag="pps")
                        for kc in range(KC):
                            nc.tensor.matmul(
                                ps[0:128 if mwid == 128 else 64, :],
                                wsv[:, sb + kc * mwid : sb + (kc + 1) * mwid],
                                xtb[:, wb + kc * 512 : wb + (kc + 1) * 512],
                                start=(kc == 0),
                                stop=(kc == KC - 1),
                            )
                        evict_fn(ps, nsl)

                    def ev_p1(ps, nsl):
                        sl = slice(nsl * 512, (nsl + 1) * 512)
                        bias = bstack["P1"]
                        nc.scalar.activation(
                            buf2[0:64, sl], ps[0:64, :], AFT.Identity,
                            bias=bias[0:64, :], scale=1.0,
                        )
                        nc.vector.scalar_tensor_tensor(
                            out=buf1[64:128, sl],
                            in0=ps[64:128, :],
                            scalar=1.0,
                            in1=bias[64:128, 0:1].broadcast_to([64, 512]),
                            op0=ALU.mult,
                            op1=ALU.add,
                        )
                        nc.gpsimd.dma_start(buf1[0:64, sl], buf1[64:128, sl])

                    def ev_p4(ps, nsl):
                        sl = slice(nsl * 512, (nsl + 1) * 512)
                        nc.scalar.activation(
                            vtsb[:, sl], ps[0:64, :], AFT.Identity,
                            bias=bstack["P4"][:], scale=1.0,
                        )

                    def ev_p3(ps, nsl):
                        bias = bstack["P3"]
                        lo = 1024 + (S - 1 - (nsl * 512 + 511))
                        nc.scalar.activation(
                            buf4[:, lo : lo + 512],
                            rev_free(ps[0:64, :], 512),
                            AFT.Identity,
                            bias=bias[0:64, :], scale=1.0,
                        )
                        nc.vector.scalar_tensor_tensor(
                            out=buf2[64:128, 1024 + nsl * 512 : 1024 + (nsl + 1) * 512],
                            in0=ps[64:128, :],
                            scalar=1.0,
                            in1=bias[64:128, 0:1].broadcast_to([64, 512]),
                            op0=ALU.mult,
                            op1=ALU.add,
                        )

                    def emit_strips(it):
                        pm0 = 1920 - 128 * it
                        for ci, (c0, w) in enumerate(CH5):
                            sp = stpsum.tile([128, 448], F32, name="sps", tag="stp")
                            nc.tensor.matmul(
                                sp[:, 0:w],
                                buf2[0:64, it * 128 : (it + 1) * 128],
                                buf4[:, pm0 + c0 : pm0 + c0 + w],
                                start=True,
                                stop=True,
                                skip_group_check=True,
                            )
                            dst = strips[:, it * WSTR + c0 : it * WSTR + c0 + w]
                            if (it + ci) % 2 == 0:
                                nc.vector.tensor_copy(dst, sp[:, 0:w])
                            else:
                                nc.scalar.activation(dst, sp[:, 0:w], AFT.Copy)

                    def emit_c2pn(jbp, ih):
                        sap = strips[:].copy()
                        _set_ap(
                            sap,
                            [(WSROW - 1, 128), (WSTR, 8), (1, 256)],
                            ih * 8 * WSTR + jbp * 256 + 127,
                        )
                        nc.sync.dma_start(cpv[(2 * jbp + ih) % 3][:], sap)

                    # A1: pos_x -> buf4 (kr reversed) + buf2[64:] (qr)
                    # (x slices 0-1's transposes fill A1's DMA-starved tail)
                    for nsl in range(4):
                        if nsl == 2:
                            M[2] = load_octo(x_d, 0)
                            for half, w in enumerate(("Wq", "Wk")):
                                stack_half(load_w(w), wst["P1"], half)
                            stack_half(load_w("Wv"), wst["P4"], None)
                        for q in range(4):
                            transpose_quad(
                                M[nsl // 2], nsl * 4 + q, (nsl % 2) * 4 + q
                            )
                        if nsl == 3:
                            for q in range(4):
                                transpose_quad(M[2], q, q)
                        project_slice("P3", nsl, ev_p3, 128)
                        if nsl == 3:
                            for q in range(4):
                                transpose_quad(M[2], 4 + q, 4 + q)
                    nc.vector.tensor_copy(
                        buf4[:, 0:1024],
                        buf4[:, 1024:1025].broadcast_to([64, 1024]),
                    )
                    nc.vector.tensor_copy(
                        buf4[:, 3072:4096],
                        buf4[:, 3071:3072].broadcast_to([64, 1024]),
                    )
                    nc.vector.tensor_copy(
                        buf2[64:128, 0:1024],
                        buf2[64:128, 1024:1025].broadcast_to([64, 1024]),
                    )
                    nc.vector.tensor_copy(
                        buf2[64:128, 3072:4096],
                        buf2[64:128, 3071:3072].broadcast_to([64, 1024]),
                    )

                    # padding mask -> [128, NB] bias columns via PE transpose
                    # (needed only by the exp in phase B)
                    mrawi = consts.tile([32, 128], I32, name="mrawi")
                    nc.vector.memset(mrawi[:], 0)
                    mview = mask_d[:].copy()
                    _set_ap(mview, [(128, NB), (1, 128)], 0)
                    nc.scalar.dma_start(mrawi[0:NB, :], mview)
                    mrawf = consts.tile([32, 128], BF16, name="mrawf")
                    nc.vector.tensor_copy(mrawf[:], mrawi[:])
                    mps = tpsum.tile([128, 32], BF16, name="mps", tag="tps")
                    nc.tensor.matmul(
                        mps[:], mrawf[:], identb[0:32, 0:32], is_transpose=True
                    )
                    nc.vector.tensor_scalar_mul(maskb[:], mps[:, 0:NB], float(NEG))

                    # A2: x -> q^T, k^T, v^T + interleaved full-width strips
                    # (slices 0-1 already transposed during A1)
                    for nsl in range(4):
                        if nsl == 0:
                            M[3] = load_octo(x_d, 1)
                        if nsl >= 2:
                            for q in range(4):
                                transpose_quad(
                                    M[3], nsl * 4 + q, (nsl % 2) * 4 + q
                                )
                        project_slice("P1", nsl, ev_p1, 128)
                        project_slice("P4", nsl, ev_p4, DH)
                        for it in range(nsl * 4, nsl * 4 + 4):
                            emit_strips(it)
                        if nsl == 1:
                            emit_c2pn(0, 0)
                            emit_c2pn(1, 0)

                    # V tiles (batched transposes -> vtil bf16 + ones col)
                    for g in range(4):
                        vp = tpsum.tile([128, 512], BF16, name="vps", tag="tps")
                        for q in range(4):
                            jbv = g * 4 + q
                            nc.tensor.matmul(
                                vp[:, q * 64 : q * 64 + 64],
                                vtsb[:, jbv * 128 : (jbv + 1) * 128],
                                identb[0:64, 0:64],
                                is_transpose=True,
                                skip_group_check=True,
                            )
                        o = vtil[:].copy()
                        _set_ap(
                            o,
                            [(NB * (DH + 1), 128), (DH + 1, 4), (1, DH)],
                            (g * 4) * (DH + 1),
                        )
                        i = vp[:].copy()
                        _set_ap(i, [(512, 128), (64, 4), (1, 64)], 0)
                        nc.vector.tensor_copy(o, i)
                    vones = vtil[:].copy()
                    _set_ap(vones, [(NB * (DH + 1), 128), (DH + 1, NB), (1, 1)], DH)
                    oview = ones_col[:].copy()
                    _set_ap(oview, [(1, 128), (0, NB), (0, 1)], 0)
                    nc.vector.tensor_copy(vones, oview)

                # ======================= attention ========================
                with (
                    tc.tile_pool(name="bpool", bufs=1) as bpool,
                    tc.tile_pool(name="skpool", bufs=2) as skpool,
                    tc.tile_pool(name="scpsum", bufs=3, space="PSUM") as scpsum,
                    tc.tile_pool(name="wps2", bufs=2, space="PSUM") as wps2,
                ):
                    expst = bpool.tile([128, 16384], BF16, name="expst")
                    pcab = [
                        bpool.tile([128, WSTR], BF16, name=f"pcab{i}")
                        for i in range(3)
                    ]
                    p2ctv = [
                        bpool.tile([128, 2048], BF16, name=f"p2ct{i}")
                        for i in range(3)
                    ]
                    def p2cs(jb):
                        su0 = 1920 - jb * 128
                        for ci, (c0, w) in enumerate(CH5):
                            pp = wps2.tile([128, 512], F32, name="pps2", tag="wps")
                            nc.tensor.matmul(
                                pp[:, 0:w],
                                buf1[64:128, jb * 128 : (jb + 1) * 128],
                                buf2[64:128, su0 + c0 : su0 + c0 + w],
                                start=True,
                                stop=True,
                                skip_group_check=True,
                            )
                            dst = pcab[jb % 3][:, c0 : c0 + w]
                            if ci % 2 == 0:
                                nc.vector.tensor_copy(dst, pp[:, 0:w])
                            else:
                                nc.scalar.activation(dst, pp[:, 0:w], AFT.Copy)

                    def p2ct_issue(jb):
                        sap = pcab[jb % 3][:].copy()
                        _set_ap(sap, [(WSTR - 1, 128), (1, 2048)], 128)
                        nc.gpsimd.dma_start(p2ctv[jb % 3][:], sap)

                    def score_tile(jb, ih):
                        jj, jbp, g = jb % 2, jb // 2, jb // 4
                        sc = scpsum.tile([128, 1024], F32, name="sc", tag="sc")
                        for nsl in (0, 1):
                            nc.tensor.matmul(
                                sc[:, nsl * 512 : (nsl + 1) * 512],
                                buf1[0:64, jb * 128 : (jb + 1) * 128],
                                buf2[
                                    0:64,
                                    ih * 1024 + nsl * 512 : ih * 1024 + (nsl + 1) * 512,
                                ],
                                start=True,
                                stop=False,
                                skip_group_check=True,
                            )
                        cp = cpv[(2 * jbp + ih) % 3]
                        for t in range(8):
                            c0 = t * 256 + jj * 128
                            # transpose-inject: out[m,n] = cp[n,m] via identity rhs
                            nc.tensor.matmul(
                                sc[:, t * 128 : (t + 1) * 128],
                                cp[:, c0 : c0 + 128],
                                identb[:],
                                start=False,
                                stop=False,
                                skip_group_check=True,
                            )
                        pt = p2ctv[jb % 3]
                        for nsl in (0, 1):
                            nc.tensor.matmul(
                                sc[:, nsl * 512 : (nsl + 1) * 512],
                                identb[:],
                                pt[:, ih * 1024 + nsl * 512 : ih * 1024 + (nsl + 1) * 512],
                                start=False,
                                stop=True,
                                skip_group_check=True,
                            )
                        dst = expst[
                            :,
                            (g % 2) * 8192
                            + (jb % 4) * 2048
                            + ih * 1024 : (g % 2) * 8192
                            + (jb % 4) * 2048
                            + (ih + 1) * 1024,
                        ]
                        nc.scalar.activation(
                            dst, sc[:], AFT.Exp,
                            bias=maskb[:, jb : jb + 1], scale=SCALE,
                        )

                    def av_pass(g):
                        for nsl in range(4):
                            avp = wps2.tile([65, 512], F32, name="avp", tag="wps")
                            for qq in range(4):
                                jbq = g * 4 + qq
                                nc.tensor.matmul(
                                    avp[:],
                                    vtil[:, jbq * (DH + 1) : (jbq + 1) * (DH + 1)],
                                    expst[
                                        :,
                                        (g % 2) * 8192
                                        + qq * 2048
                                        + nsl * 512 : (g % 2) * 8192
                                        + qq * 2048
                                        + (nsl + 1) * 512,
                                    ],
                                    start=(qq == 0),
                                    stop=(qq == 3),
                                    skip_group_check=True,
                                )
                            sl = slice(nsl * 512, (nsl + 1) * 512)
                            if g == 0:
                                nc.scalar.activation(outacc[:, sl], avp[:], AFT.Copy)
                            else:
                                nc.vector.scalar_tensor_tensor(
                                    out=outacc[:, sl],
                                    in0=avp[:],
                                    scalar=1.0,
                                    in1=outacc[:, sl],
                                    op0=ALU.mult,
                                    op1=ALU.add,
                                )

                    emit_c2pn(0, 1)
                    p2cs(0)
                    p2ct_issue(0)
                    p2cs(1)
                    p2ct_issue(1)
                    for jb in range(16):
                        if jb % 2 == 0 and jb >= 2 and jb // 2 + 1 < 8:
                            emit_c2pn(jb // 2 + 1, 0)
                        if jb + 2 < 16:
                            p2cs(jb + 2)
                            p2ct_issue(jb + 2)
                        if jb % 4 == 0 and jb > 0:
                            av_pass(jb // 4 - 1)
                        score_tile(jb, 0)
                        if jb % 2 == 1 and jb // 2 + 1 < 8:
                            emit_c2pn(jb // 2 + 1, 1)
                        score_tile(jb, 1)

                    # ---- last AV group software-pipelined with the final
                    # transpose+normalize (transposes of slice n run while the
                    # DVE accumulates slice n+1)
                    def av3_chunk(nsl):
                        avp = wps2.tile([65, 512], F32, name="avp", tag="wps")
                        for qq in range(4):
                            nc.tensor.matmul(
                                avp[:],
                                vtil[:, (12 + qq) * (DH + 1) : (13 + qq) * (DH + 1)],
                                expst[
                                    :,
                                    8192 + qq * 2048 + nsl * 512 :
                                    8192 + qq * 2048 + (nsl + 1) * 512,
                                ],
                                start=(qq == 0),
                                stop=(qq == 3),
                                skip_group_check=True,
                            )
                        sl = slice(nsl * 512, (nsl + 1) * 512)
                        nc.vector.scalar_tensor_tensor(
                            out=outacc[:, sl],
                            in0=avp[:],
                            scalar=1.0,
                            in1=outacc[:, sl],
                            op0=ALU.mult,
                            op1=ALU.add,
                        )

                    def norm_chunk(nsl):
                        for t in range(nsl * 4, nsl * 4 + 4):
                            fp = wps2.tile([128, 512], F32, name="fps", tag="wps")
                            nc.tensor.matmul(
                                fp[:, 0:65],
                                outacc[:, t * 128 : (t + 1) * 128],
                                ident32[0:65, 0:65],
                                is_transpose=True,
                                skip_group_check=True,
                            )
                            rcol = skpool.tile([128, 1], F32, name="rcol", tag="rcol")
                            nc.vector.reciprocal(rcol[:], fp[:, 64:65])
                            nc.vector.tensor_scalar_mul(
                                outsb[:, t * DH : (t + 1) * DH], fp[:, 0:64], rcol[:]
                            )
                        oap = out_d[:].copy()
                        _set_ap(
                            oap,
                            [(DH, 128), (128 * DH, 4), (1, DH)],
                            nsl * 4 * 128 * DH,
                        )
                        nc.sync.dma_start(
                            oap, outsb[:, nsl * 4 * DH : (nsl + 1) * 4 * DH]
                        )

                    av3_chunk(0)
                    for nsl in range(1, 4):
                        av3_chunk(nsl)
                        norm_chunk(nsl - 1)
                    norm_chunk(3)

                    if debug:
                        dbg = {
                            "dbg_strips": ([128, 4352], BF16, strips[:, 0:4352]),
                            "dbg_cp": ([128, 2048], BF16, cpv[0][:]),
                            "dbg_pc": ([128, 2176], BF16, pcab[0][:]),
                            "dbg_p2ct": ([128, 2048], BF16, p2ctv[0][:]),
                            "dbg_exp": ([128, 2048], BF16, expst[:, 0:2048]),
                            "dbg_buf1": ([128, 2048], F32, buf1[:].bitcast(F32)),
                            "dbg_buf2a": ([64, 2048], F32, buf2[0:64, 0:2048].bitcast(F32)),
                            "dbg_buf2b": ([64, 4096], F32, buf2[64:128, :].bitcast(F32)),
                            "dbg_buf4": ([64, 4096], F32, buf4[:].bitcast(F32)),
                        }
                        for nm, (shp, dt, src) in dbg.items():
                            dd = nc.dram_tensor(nm, shp, dt, kind="ExternalOutput")
                            nc.sync.dma_start(dd[:], src)

    if split_waits:
        _split_excess_waits(nc)
    nc.finalize()
    return nc


_NC_CACHE = None


def _get_nc():
    global _NC_CACHE
    if _NC_CACHE is None:
        _NC_CACHE = build_nc()
    return _NC_CACHE


def kernel(**inputs):
    from concourse.bass_utils import run_bass_kernel_spmd

    nc = _get_nc()
    in_maps = []
    for b in range(B):
        m = {
            "x": np.ascontiguousarray(inputs["x"][b]),
            "pos_x": np.ascontiguousarray(inputs["pos_x"][b]),
            "padding_mask": np.ascontiguousarray(inputs["padding_mask"][b]),
        }
        for w in ("Wq", "Wk", "Wv", "Wqr", "Wkr"):
            m[w] = np.ascontiguousarray(inputs[w])
        for bn in ("bq", "bk", "bv", "bqr", "bkr"):
            m[bn] = np.ascontiguousarray(inputs[bn])
        in_maps.append(m)
    res = run_bass_kernel_spmd(nc, in_maps, core_ids=list(range(B)))
    return np.stack([r["out"] for r in res.results])
